# revision 1
# baseline (speedup 1.0000x reference)
"""Trainium2 Bass kernel for nested triangle multiplicative update (GNN message passing).

Strategy: data-parallel over nodes N=1024 across 8 cores (128 nodes/core).
Per core everything stays on-chip: translations gathered via gpsimd ap_gather,
distances via broadcast-AP vector ops (node-per-partition), RBF via PE matmul
exponent (a*D + b*D^2 + c) + one ACT Exp pass, d3 via bf16 matmul, triangle
contraction via TT-multiply + grouped reduce, layernorms via PE-stats.
"""
import sys

sys.path.insert(0, "/opt/trn_rl_repo")

import numpy as np

import concourse.bacc as bacc
import concourse.bass as bass
import concourse.mybir as mybir
import concourse.tile as tile
from concourse.bass import AP
from concourse.bass_utils import run_bass_kernel_spmd

F32 = mybir.dt.float32
BF16 = mybir.dt.bfloat16
I16 = mybir.dt.int16

N, K, C_S, C_Z, C_G, R = 1024, 32, 384, 128, 16, 64
NCORES = 8
NN = N // NCORES          # nodes per core = 128
NK = NN * K               # edges per core = 4096
NCH = NK // 128           # 128-row chunks of edges = 32
D_MAX, EPS_LN = 20.0, 1e-5
SIGMA = D_MAX / R                     # 0.3125
MU = np.linspace(0.0, D_MAX, R)      # spacing 20/63


def ts(i, n):
    return slice(i * n, (i + 1) * n)


def bc(ap, pos, rep):
    """Insert a broadcast (step-0) dim of length rep at free-dim position pos."""
    newap = list(ap.ap)
    newap.insert(pos, [0, rep])
    return AP(ap.tensor, ap.offset, newap)


def build_nc():
    nc = bacc.Bacc("TRN2", target_bir_lowering=False, debug=False)
    P = lambda name, shape, dt: nc.declare_dram_parameter(name, list(shape), dt, isOutput=False)

    ef_d = P("ef", [NK, C_Z], F32)
    dstw_d = P("dstw", [128, K], I16)
    ntabT_d = P("ntabT", [4, N], F32)
    nfTa_d = P("nfTa", [C_S + 1, NN], F32)
    wnlra_d = P("wnlra", [C_S + 1, 2 * C_G], F32)
    wdg_d = P("wdg", [C_G * C_G, C_Z], F32)
    bdg_d = P("bdg", [C_Z], F32)
    expL4_d = P("expL4", [4, 128], F32)
    cvec_d = P("cvec", [128], F32)
    wdp2_d = P("wdp2", [128, C_Z], BF16)
    bdp_d = P("bdp", [C_Z], F32)
    wgeg_d = P("wgeg", [C_Z, C_Z], BF16)
    wgep_d = P("wgep", [C_Z, C_Z], BF16)
    wgog_d = P("wgog", [C_Z, C_Z], BF16)
    beg_d = P("beg", [C_Z], F32)
    bep_d = P("bep", [C_Z], F32)
    bog_d = P("bog", [C_Z], F32)
    wglo_d = P("wglo", [C_Z, C_Z], BF16)
    blo2_d = P("blo2", [C_Z], F32)
    idf_d = P("idf", [128, 128], F32)
    idb_d = P("idb", [128, 128], BF16)
    sel_d = P("sel", [128, 15], F32)
    onesB_d = P("onesB", [97, 128], F32)
    out_d = nc.declare_dram_parameter("out", [NK, C_Z], F32, isOutput=True)

    mult, add_op, sub_op, maxop = (mybir.AluOpType.mult, mybir.AluOpType.add,
                                   mybir.AluOpType.subtract, mybir.AluOpType.max)
    AF = mybir.ActivationFunctionType
    AX = mybir.AxisListType

    with tile.TileContext(nc) as tc:
        with (
            tc.tile_pool(name="big", bufs=1) as big,
            tc.tile_pool(name="wk", bufs=2) as wk,
            tc.tile_pool(name="wk1", bufs=1) as wk1,
            tc.tile_pool(name="ps", bufs=2, space="PSUM") as ps,
            tc.tile_pool(name="ps_ex", bufs=2, space="PSUM") as ps_ex,
            tc.tile_pool(name="ps_d3", bufs=2, space="PSUM") as ps_d3,
            tc.tile_pool(name="ps_st", bufs=1, space="PSUM") as ps_st,
        ):
            # ---------- persistent tiles + loads ----------
            ef_t = big.tile([128, NCH, C_Z], F32)          # ef rows 128c+p
            nc.sync.dma_start(
                ef_t[:, :, :],
                AP(ef_d.ap().tensor, 0, [[C_Z, 128], [128 * C_Z, NCH], [1, C_Z]]),
            )
            dstw_t = big.tile([128, K], I16)
            nc.sync.dma_start(dstw_t[:], dstw_d.ap()[:, :])
            tabrep_t = big.tile([128, N, 1], F32)
            nc.sync.dma_start(
                tabrep_t[:, :, 0],
                AP(ntabT_d.ap().tensor, 0, [[0, 32], [N, 4], [1, N]]),
            )
            nfTa_t = big.tile([128, 4, NN], F32)           # chunks of 128 rows (385 -> 4 chunks, last partial)
            for c in range(3):
                nc.sync.dma_start(nfTa_t[:, c, :], nfTa_d.ap()[ts(c, 128), :])
            nc.sync.dma_start(nfTa_t[0:1, 3, :], nfTa_d.ap()[384:385, :])
            wnlra_t = big.tile([128, 4, 2 * C_G], F32)
            for c in range(3):
                nc.sync.dma_start(wnlra_t[:, c, :], wnlra_d.ap()[ts(c, 128), :])
            nc.sync.dma_start(wnlra_t[0:1, 3, :], wnlra_d.ap()[384:385, :])
            wdg_t = big.tile([128, 2, C_Z], F32)
            nc.sync.dma_start(wdg_t[:, 0, :], wdg_d.ap()[0:128, :])
            nc.sync.dma_start(wdg_t[:, 1, :], wdg_d.ap()[128:256, :])
            expL4_t = big.tile([4, 128], F32)
            nc.sync.dma_start(expL4_t[:], expL4_d.ap()[:, :])
            wdp2_t = big.tile([128, C_Z], BF16)
            nc.sync.dma_start(wdp2_t[:], wdp2_d.ap()[:, :])
            wgeg_t = big.tile([C_Z, C_Z], BF16)
            nc.sync.dma_start(wgeg_t[:], wgeg_d.ap()[:, :])
            wgep_t = big.tile([C_Z, C_Z], BF16)
            nc.sync.dma_start(wgep_t[:], wgep_d.ap()[:, :])
            wgog_t = big.tile([C_Z, C_Z], BF16)
            nc.sync.dma_start(wgog_t[:], wgog_d.ap()[:, :])
            wglo_t = big.tile([C_Z, C_Z], BF16)
            nc.sync.dma_start(wglo_t[:], wglo_d.ap()[:, :])
            idf_t = big.tile([128, 128], F32)
            nc.sync.dma_start(idf_t[:], idf_d.ap()[:, :])
            idb_t = big.tile([128, 128], BF16)
            nc.sync.dma_start(idb_t[:], idb_d.ap()[:, :])
            sel_t = big.tile([128, 15], F32)
            nc.sync.dma_start(sel_t[:], sel_d.ap()[:, :])
            onesB_t = big.tile([97, 128], F32)
            nc.sync.dma_start(onesB_t[:], onesB_d.ap()[:, :])

            def colvec(name_d):
                t = big.tile([128, 1], F32, tag=f"cv_{name_d.name}")
                nc.sync.dma_start(t[:, 0:1], AP(name_d.ap().tensor, 0, [[1, 128], [1, 1]]))
                return t
            bdg_t, cvec_t, bdp_t = colvec(bdg_d), colvec(cvec_d), colvec(bdp_d)
            beg_t, bep_t, bog_t, blo2_t = colvec(beg_d), colvec(bep_d), colvec(bog_d), colvec(blo2_d)

            # ---------- S1: gather translations ----------
            gat_t = big.tile([128, 512, 1], F32)
            nc.gpsimd.ap_gather(gat_t[:, :, :], tabrep_t[:, :, :], dstw_t[:, :],
                                channels=128, num_elems=N, d=1, num_idxs=512)
            tn_t = big.tile([128, 4, K], F32)              # [node, coord(3 used), k]
            for g in range(8):
                for c in range(3):
                    src = gat_t[16 * g + c:16 * g + c + 1, :, 0].rearrange(
                        "p (a b) -> p a b", a=16)
                    nc.sync.dma_start(tn_t[16 * g:16 * (g + 1), c, :], src)

            # ---------- S2: dist^2 and D (all nodes at once) ----------
            dif_t = big.tile([128, 3, K, K], F32)
            in_j = bc(tn_t[:, 0:3, :], 2, K)      # [p, c, i(bcast), j]
            in_i = bc(tn_t[:, 0:3, :], 3, K)      # [p, c, i, j(bcast)]
            nc.vector.tensor_tensor(dif_t[:, :, :, :], in_j, in_i, op=sub_op)
            d3k = dif_t[:, :, :, :].rearrange("p a b c -> p (a b c)")
            nc.vector.tensor_tensor(d3k, d3k, d3k, op=mult)
            d2c_t = big.tile([128, K * K], F32)
            difap = dif_t[:, :, :, :]
            nc.vector.tensor_reduce(
                d2c_t[:, :],
                AP(difap.tensor, difap.offset, [difap.ap[0], [1, K * K], [K * K, 3]]),
                axis=AX.X, op=add_op)
            nc.vector.tensor_scalar_max(d2c_t[:, :], d2c_t[:, :], 0.0)
            dD_t = big.tile([128, K * K], F32)
            nc.scalar.sqrt(dD_t[:, :], d2c_t[:, :])

            # ---------- S4: nl/nr -> outer -> gate3 ----------
            nlnr_ps = ps.tile([128, 2 * C_G], F32, tag="m")
            for c in range(4):
                kdim = 128 if c < 3 else 1
                nc.tensor.matmul(nlnr_ps[:, :], nfTa_t[0:kdim, c, :], wnlra_t[0:kdim, c, :],
                                 start=(c == 0), stop=(c == 3))
            nlnr_t = wk.tile([128, 2 * C_G], F32, tag="nlnr")
            nc.vector.tensor_copy(nlnr_t[:, :], nlnr_ps[:, :])
            outer_t = wk.tile([128, C_G * C_G], F32, tag="outer")
            for a in range(C_G):
                nc.vector.tensor_scalar_mul(outer_t[:, ts(a, C_G)], nlnr_t[:, C_G:2 * C_G],
                                            nlnr_t[:, a:a + 1])
            oT_sb = wk.tile([128, 2, 128], F32, tag="oT")
            for h in range(2):
                oT_ps = ps.tile([128, 128], F32, tag="m")
                nc.tensor.transpose(oT_ps[:, :], outer_t[:, ts(h, 128)], idf_t[:, :])
                nc.vector.tensor_copy(oT_sb[:, h, :], oT_ps[:, :])
            g3_ps = ps.tile([128, 128], F32, tag="m")
            for h in range(2):
                nc.tensor.matmul(g3_ps[:, :], wdg_t[:, h, :], oT_sb[:, h, :],
                                 start=(h == 0), stop=(h == 1))
            g3_t = wk.tile([128, NN], BF16, tag="g3")
            nc.scalar.activation(g3_t[:, :], g3_ps[:, :], AF.Sigmoid, bias=bdg_t[:, :])

            # ---------- S5: LN(ef) stats + xhat + transpose ----------
            ms_t = wk.tile([128, NCH], F32, tag="ms")
            ss_t = wk.tile([128, NCH], F32, tag="ss")
            for c in range(NCH):
                nc.vector.tensor_reduce(ms_t[:, c:c + 1], ef_t[:, c, :], axis=AX.X, op=add_op)
                scr = wk.tile([128, C_Z], BF16, tag="scr")
                nc.scalar.activation(scr[:, :], ef_t[:, c, :], AF.Square,
                                     accum_out=ss_t[:, c:c + 1])
            mm_t = wk.tile([128, NCH], F32, tag="lnst")
            nc.vector.tensor_scalar_mul(mm_t[:, :], ms_t[:, :], 1.0 / C_Z)
            ex2_t = wk.tile([128, NCH], F32, tag="lnst2")
            nc.vector.tensor_scalar_mul(ex2_t[:, :], ss_t[:, :], 1.0 / C_Z)
            var_t = wk.tile([128, NCH], F32, tag="lnst3")
            nc.vector.tensor_tensor(var_t[:, :], mm_t[:, :], mm_t[:, :], op=mult)
            nc.vector.tensor_tensor(var_t[:, :], ex2_t[:, :], var_t[:, :], op=sub_op)
            nc.vector.tensor_scalar_add(var_t[:, :], var_t[:, :], EPS_LN)
            inv_t = wk.tile([128, NCH], F32, tag="lnst4")
            nc.vector.reciprocal(inv_t[:, :], var_t[:, :])
            rstd_t = wk.tile([128, NCH], F32, tag="lnst5")
            nc.scalar.sqrt(rstd_t[:, :], inv_t[:, :])
            mrs_t = wk.tile([128, NCH], F32, tag="lnst6")
            nc.vector.tensor_tensor(mrs_t[:, :], mm_t[:, :], rstd_t[:, :], op=mult)

            xT_t = big.tile([C_Z, NK], BF16)
            for c4 in range(NCH // 4):
                xp_ps = ps.tile([128, 4, 128], BF16, tag="m")
                for j in range(4):
                    c = 4 * c4 + j
                    xh = wk.tile([128, C_Z], BF16, tag="xh")
                    nc.vector.tensor_scalar(xh[:, :], ef_t[:, c, :],
                                            rstd_t[:, c:c + 1], mrs_t[:, c:c + 1],
                                            op0=mult, op1=sub_op)
                    nc.tensor.transpose(xp_ps[:, j, :], xh[:, :], idb_t[:, :])
                nc.scalar.copy(xT_t[:, ts(c4, 512)],
                               xp_ps[:, :, :].rearrange("p a b -> p (a b)"))

            # ---------- S6: e2 gates ----------
            sge_t = big.tile([C_Z, NK], BF16)
            pep_t = big.tile([C_Z, NK], BF16)
            ogs_t = big.tile([C_Z, NK], BF16)
            for (w_t, b_t, fn, dst) in ((wgeg_t, beg_t, AF.Sigmoid, sge_t),
                                        (wgep_t, bep_t, AF.Identity, pep_t),
                                        (wgog_t, bog_t, AF.Sigmoid, ogs_t)):
                for q in range(8):
                    e_ps = ps.tile([128, 512], F32, tag="m")
                    nc.tensor.matmul(e_ps[:, :], w_t[:, :], xT_t[:, ts(q, 512)],
                                     start=True, stop=True)
                    nc.scalar.activation(dst[:, ts(q, 512)], e_ps[:, :], fn, bias=b_t[:, :])
            ee_t = big.tile([C_Z, NK], BF16)
            nc.vector.tensor_tensor(ee_t[:, :], sge_t[:, :], pep_t[:, :], op=mult)
            g3b = bc(g3_t[:, :], 2, K)          # [z, n, K] step-0 over K
            eeap = ee_t[:, :]
            ee3 = AP(eeap.tensor, eeap.offset, [eeap.ap[0], [K, NN], [1, K]])
            nc.vector.tensor_tensor(ee3, ee3, g3b, op=mult)
            see_t = wk.tile([128, NN], F32, tag="see")
            nc.vector.tensor_reduce(see_t[:, :], ee3, axis=AX.X, op=add_op)
            biasn_t = wk.tile([128, NN], F32, tag="biasn")
            nc.vector.tensor_scalar_mul(biasn_t[:, :], see_t[:, :], bdp_t[:, :])

            # ---------- S7: per node-pair RBF -> d3 -> triangle contract ----------
            upd_t = big.tile([C_Z, NK], F32)
            for q in range(NN // 2):
                # stage the pair's (D, D^2) rows interleaved at base partition 0
                rhsq = wk.tile([4, 1024], F32, tag="rhsq")
                nc.sync.dma_start(rhsq[0:2, :], dD_t[ts(q, 2), :])
                nc.sync.dma_start(rhsq[2:4, :], d2c_t[ts(q, 2), :])
                for h in range(2):
                    ex_ps = ps_ex.tile([128, 512], F32, tag="ex")
                    nc.tensor.matmul(ex_ps[:, :], expL4_t[:, :], rhsq[:, ts(h, 512)],
                                     start=True, stop=True)
                    rbf = wk.tile([128, 512], BF16, tag="rbf")
                    nc.scalar.activation(rbf[:, :], ex_ps[:, :], AF.Exp, bias=cvec_t[:, :])
                    for u in range(2):
                        n = 2 * q + u
                        d3_ps = ps_d3.tile([128, 512], F32, tag="d3")
                        nc.tensor.matmul(d3_ps[:, :], wdp2_t[ts(u, 64), :], rbf[ts(u, 64), :],
                                         start=True, stop=True)
                        prod = wk.tile([128, 512], BF16, tag="prod")
                        eesl = bc(ee_t[:, ts(n, K)], 1, 16)
                        if q % 2 == 0:
                            d3s = wk.tile([128, 512], BF16, tag="d3s")
                            nc.scalar.copy(d3s[:, :], d3_ps[:, :])
                            nc.vector.tensor_tensor(
                                prod[:, :].rearrange("p (a b) -> p a b", a=16),
                                d3s[:, :].rearrange("p (a b) -> p a b", a=16),
                                eesl, op=mult)
                        else:
                            nc.vector.tensor_tensor(
                                prod[:, :].rearrange("p (a b) -> p a b", a=16),
                                d3_ps[:, :].rearrange("p (a b) -> p a b", a=16),
                                eesl, op=mult)
                        nc.vector.tensor_reduce(
                            upd_t[:, n * K + 16 * h: n * K + 16 * h + 16],
                            prod[:, :].rearrange("p (a b) -> p a b", a=16),
                            axis=AX.X, op=add_op)

            # ---------- S8: add bias term ----------
            updap = upd_t[:, :]
            upd3 = AP(updap.tensor, updap.offset, [updap.ap[0], [K, NN], [1, K]])
            nc.vector.tensor_tensor(upd3, upd3, bc(biasn_t[:, :], 2, K), op=add_op)

            # ---------- S9: LN_o via PE stats ----------
            st1_ps = ps_st.tile([8, 512], F32, tag="st1")
            st2_ps = ps_st.tile([8, 512], F32, tag="st2")
            squ_list = []
            for c in range(8):
                squ = wk.tile([128, 512], F32, tag="squ")
                nc.vector.tensor_tensor(squ[:, :], upd_t[:, ts(c, 512)], upd_t[:, ts(c, 512)],
                                        op=mult)
                squ_list.append(squ)
            for c in range(8):
                nc.tensor.matmul(st1_ps[:, :], sel_t[:, 7 - c:15 - c], upd_t[:, ts(c, 512)],
                                 start=(c == 0), stop=(c == 7))
            for c in range(8):
                nc.tensor.matmul(st2_ps[:, :], sel_t[:, 7 - c:15 - c], squ_list[c][:, :],
                                 start=(c == 0), stop=(c == 7))
            mo_t = wk1.tile([8, 512], F32, tag="mo")
            nc.vector.tensor_copy(mo_t[:, :], st1_ps[:, :])
            exo_t = wk1.tile([8, 512], F32, tag="exo")
            nc.vector.tensor_copy(exo_t[:, :], st2_ps[:, :])
            varo_t = wk1.tile([8, 512], F32, tag="varo")
            nc.vector.tensor_tensor(varo_t[:, :], mo_t[:, :], mo_t[:, :], op=mult)
            nc.vector.tensor_tensor(varo_t[:, :], exo_t[:, :], varo_t[:, :], op=sub_op)
            nc.vector.tensor_scalar_add(varo_t[:, :], varo_t[:, :], EPS_LN)
            invo_t = wk1.tile([8, 512], F32, tag="invo")
            nc.vector.reciprocal(invo_t[:, :], varo_t[:, :])
            rso_t = wk1.tile([8, 512], F32, tag="rso")
            nc.scalar.sqrt(rso_t[:, :], invo_t[:, :])
            mrso_t = wk1.tile([8, 512], F32, tag="mrso")
            nc.vector.tensor_tensor(mrso_t[:, :], mo_t[:, :], rso_t[:, :], op=mult)
            # scatter stats rows to 32-aligned partitions (bases limited to 0/32/64)
            rso3 = [wk1.tile([65, 512], F32, tag=f"rso3_{i}", name=f"rso3_{i}")
                    for i in range(3)]
            mrso3 = [wk1.tile([65, 512], F32, tag=f"mrso3_{i}", name=f"mrso3_{i}")
                     for i in range(3)]
            for c in range(8):
                nc.sync.dma_start(rso3[c // 3][32 * (c % 3):32 * (c % 3) + 1, :],
                                  rso_t[c:c + 1, :])
                nc.sync.dma_start(mrso3[c // 3][32 * (c % 3):32 * (c % 3) + 1, :],
                                  mrso_t[c:c + 1, :])

            xo_t = big.tile([C_Z, NK], BF16)
            for c in range(8):
                p0 = 32 * (c % 3)
                rb_ps = ps.tile([128, 512], F32, tag="m")
                nc.tensor.matmul(rb_ps[:, :], onesB_t[p0:p0 + 1, :],
                                 rso3[c // 3][p0:p0 + 1, :], start=True, stop=True)
                t1 = wk.tile([128, 512], F32, tag="t1")
                nc.vector.tensor_tensor(t1[:, :], upd_t[:, ts(c, 512)], rb_ps[:, :], op=mult)
                mb_ps = ps.tile([128, 512], F32, tag="m")
                nc.tensor.matmul(mb_ps[:, :], onesB_t[p0:p0 + 1, :],
                                 mrso3[c // 3][p0:p0 + 1, :], start=True, stop=True)
                nc.vector.tensor_tensor(xo_t[:, ts(c, 512)], t1[:, :], mb_ps[:, :], op=sub_op)

            # ---------- S10: final projection + out gate ----------
            outT_t = big.tile([C_Z, NK], F32)
            for q in range(8):
                f_ps = ps.tile([128, 512], F32, tag="m")
                nc.tensor.matmul(f_ps[:, :], wglo_t[:, :], xo_t[:, ts(q, 512)],
                                 start=True, stop=True)
                fo = wk.tile([128, 512], BF16, tag="fo")
                nc.scalar.activation(fo[:, :], f_ps[:, :], AF.Identity, bias=blo2_t[:, :])
                nc.vector.tensor_tensor(outT_t[:, ts(q, 512)], fo[:, :], ogs_t[:, ts(q, 512)],
                                        op=mult)

            # ---------- S11: transpose back + store ----------
            for c4 in range(8):
                op_ps = ps.tile([128, 4, 128], F32, tag="m")
                for j in range(4):
                    c = 4 * c4 + j
                    nc.tensor.transpose(op_ps[:, j, :], outT_t[:, ts(c, 128)], idf_t[:, :])
                orow = wk.tile([128, 4, 128], F32, tag="orow")
                nc.vector.tensor_copy(orow[:, :, :], op_ps[:, :, :])
                dst_ap = AP(out_d.ap().tensor, c4 * 512 * C_Z,
                            [[C_Z, 128], [128 * C_Z, 4], [1, C_Z]])
                nc.sync.dma_start(dst_ap, orow[:, :, :])

    nc.compile()
    return nc


def host_prep(inputs):
    """Build per-core input maps from full inputs (host-side sharding + param prep)."""
    nf = np.asarray(inputs["node_features"], np.float32)
    nt = np.asarray(inputs["node_trans"], np.float32)
    ef = np.asarray(inputs["edge_features"], np.float32)
    ei = np.asarray(inputs["edge_index"])
    dst = np.asarray(ei[1], np.int64).reshape(N, K)

    sig2 = SIGMA * SIGMA
    a_r = (2.0 * MU / sig2).astype(np.float32)
    b_r = np.float32(-1.0 / sig2)
    c_r = (-(MU * MU) / sig2).astype(np.float32)
    expL4 = np.zeros((4, 128), np.float32)
    expL4[0, 0:64] = a_r
    expL4[1, 64:128] = a_r
    expL4[2, 0:64] = b_r
    expL4[3, 64:128] = b_r
    cvec = np.concatenate([c_r, c_r]).astype(np.float32)

    ln_g, ln_b = np.asarray(inputs["ln_g"], np.float32), np.asarray(inputs["ln_b"], np.float32)
    lno_g, lno_b = np.asarray(inputs["lno_g"], np.float32), np.asarray(inputs["lno_b"], np.float32)
    W_eg, b_eg = np.asarray(inputs["W_eg"], np.float32), np.asarray(inputs["b_eg"], np.float32)
    W_ep, b_ep = np.asarray(inputs["W_ep"], np.float32), np.asarray(inputs["b_ep"], np.float32)
    W_og, b_og = np.asarray(inputs["W_og"], np.float32), np.asarray(inputs["b_og"], np.float32)
    W_lo, b_lo = np.asarray(inputs["W_lo"], np.float32), np.asarray(inputs["b_lo"], np.float32)

    def fold(W, b):
        return (ln_g[:, None] * W).astype(np.float32), (ln_b @ W + b).astype(np.float32)
    wgeg, beg = fold(W_eg, b_eg)
    wgep, bep = fold(W_ep, b_ep)
    wgog, bog = fold(W_og, b_og)
    wglo = (lno_g[:, None] * W_lo).astype(np.float32)
    blo2 = (lno_b @ W_lo + b_lo).astype(np.float32)

    nfTa = np.concatenate([nf.T, np.ones((1, N), np.float32)], axis=0)
    wnlra = np.concatenate(
        [np.concatenate([inputs["W_nl"], inputs["W_nr"]], axis=1),
         np.concatenate([inputs["b_nl"], inputs["b_nr"]])[None, :]], axis=0
    ).astype(np.float32)

    ntabT = np.zeros((4, N), np.float32)
    ntabT[0:3] = nt.T

    sel = np.zeros((128, 15), np.float32)
    sel[:, 7] = 1.0 / C_Z

    shared = dict(
        ntabT=ntabT, wnlra=wnlra,
        wdg=np.asarray(inputs["W_dg"], np.float32), bdg=np.asarray(inputs["b_dg"], np.float32),
        expL4=expL4, cvec=cvec,
        wdp2=np.concatenate([np.asarray(inputs["W_dp"], np.float32)] * 2, axis=0),
        bdp=np.asarray(inputs["b_dp"], np.float32),
        wgeg=wgeg, wgep=wgep, wgog=wgog, beg=beg, bep=bep, bog=bog,
        wglo=wglo, blo2=blo2,
        idf=np.eye(128, dtype=np.float32), idb=np.eye(128, dtype=np.float32),
        sel=sel, onesB=np.ones((97, 128), np.float32),
    )
    # bf16 params: run_bass_kernel_spmd feeds raw arrays; cast via jax bf16 expectation.
    import jax.numpy as jnp
    for k in ("wdp2", "wgeg", "wgep", "wgog", "wglo", "idb"):
        shared[k] = np.asarray(jnp.asarray(shared[k], jnp.bfloat16))

    in_maps = []
    for c in range(NCORES):
        nsl = slice(c * NN, (c + 1) * NN)
        esl = slice(c * NK, (c + 1) * NK)
        dstc = dst[nsl].reshape(-1)                       # [4096]
        dstw = np.zeros((128, K), np.int16)
        for g in range(8):
            blk = dstc[512 * g: 512 * (g + 1)]
            for i in range(512):
                dstw[16 * g + i % 16, i // 16] = blk[i]
        m = dict(shared)
        m["ef"] = ef[esl]
        m["dstw"] = dstw
        m["nfTa"] = np.ascontiguousarray(nfTa[:, nsl])
        in_maps.append(m)
    return in_maps


_NC_CACHE = {}


def kernel(**inputs) -> np.ndarray:
    if "nc" not in _NC_CACHE:
        _NC_CACHE["nc"] = build_nc()
    nc = _NC_CACHE["nc"]
    in_maps = host_prep(inputs)
    res = run_bass_kernel_spmd(nc, in_maps, list(range(NCORES)))
    out = np.concatenate([res.results[c]["out"] for c in range(NCORES)], axis=0)
    return out.astype(np.float32)

